# revision 10
# baseline (speedup 1.0000x reference)
"""Distance-aware masking kernel for Trainium2 (8 NeuronCores).

Computes mask[i,j,:] = W2 @ relu(W1 @ [r_i - c_j, |r_i - c_j|] + b1) + b2
for N=4096 nodes, DIM_OUT=8, sharded by rows across 8 cores (512 rows each).

Math (per hidden unit m, a_m = W1[m,3], s_m = sign(a_m)):
  u_m(i,j) = a_m*dist + alpha_m(i) - g_m(j),  h_m = relu(u_m)
with alpha = r@Wc.T + b1, g = c@Wc.T.  Let sigma = majority sign of a.
Define the uniform slot value (computable with ONE tensor_tensor subtract
whose operand order depends on sigma, plus per-partition scalars):
  t_p = sigma*(|a_m| D) + s_m*... arranged so that slots with s_m == sigma
  hold u_m and the (<=1) minority slot holds -u_m'.
Then relu is a single fused tensor_scalar (add alpha-scalar, max clamp):
clamp=0 gives relu(u) / relu(-u); the pad slot (clamp=-BIG) passes -u_m'
linearly and the block-diagonal mix matmul adds W2[:,m']*u_m' via a
-W2[:,m'] column, using relu(u) = u + relu(-u).

Per-core pipeline, 64 iterations of (block b: 32 rows) x (jp: 1024 cols):
  PE   : S~ = a_m^2 (dist^2+eps) as K=15 bf16 split-pair matmul -> PSUM
  ACT  : t_d = sqrt(S~) -> f16 SBUF  (+ share of PSUM->SBUF output copies)
  DVE  : tmp = t_d - s*g (f16 2x) ; h = max(tmp + s_alpha, clamp) (f16 4x)
  Pool : share of the tmp subtract (SBUF-only engine)
  PE   : 4 block-diagonal mix matmuls (f16) -> PSUM [q=8*dl+o, (w,j)]
  ACT/DVE: copy PSUM f32 -> f16 SBUF, DMA out (f16 scratch, 2KB chunks)
Host: assemble f32 output, add b2, patch exact diagonal.
"""

import sys

sys.path.insert(0, "/opt/trn_rl_repo")

import numpy as np
import ml_dtypes

N = 4096
N_CORES = 8
ROWS = N // N_CORES          # 512 rows per core
IB = 32                      # i-rows per block (x4 slots = 128 partitions)
NB = ROWS // IB              # 16 blocks
JP = 1024                    # j-columns per iteration
NJ = N // JP                 # 4 j-chunks
EPS = 2e-3                   # dist^2 floor; covers bf16-split basis error
DIM = 3
DIM_OUT = 8
NEG_BIG = -3.0e38            # clamp sentinel: pass-through (no relu)

POOL_TT = 0                  # columns of the subtract done on GpSimd/Pool
POOL_TS = True               # run the fused add/max on GpSimd/Pool

_BF = ml_dtypes.bfloat16
_F16 = np.float16

_CACHE = {}


def _split2(x):
    hi = x.astype(_BF)
    lo = (x - hi.astype(np.float32)).astype(_BF)
    return hi, lo


def _build_program(sigma):
    """Build + compile the SPMD Bass program (sigma = majority sign of a)."""
    import concourse.bass as bass  # noqa: F401
    import concourse.mybir as mybir
    import concourse.tile as tile
    from concourse import bacc

    nc = bacc.Bacc("TRN2", target_bir_lowering=False, num_devices=N_CORES)

    f32 = mybir.dt.float32
    f16 = mybir.dt.float16
    bf16 = mybir.dt.bfloat16
    AL = mybir.AluOpType

    sv_lhsT = nc.dram_tensor("sv_lhsT", [15, NB * 128], bf16, kind="ExternalInput").ap()
    sv_rhs = nc.dram_tensor("sv_rhs", [15, N], bf16, kind="ExternalInput").ap()
    gtab = nc.dram_tensor("gtab", [128, N], f16, kind="ExternalInput").ap()
    salpha = nc.dram_tensor("salpha", [128, NB], f32, kind="ExternalInput").ap()
    clamp = nc.dram_tensor("clamp", [128, 1], f32, kind="ExternalInput").ap()
    mixw = nc.dram_tensor("mixw", [128, 128], f16, kind="ExternalInput").ap()
    scratch = nc.dram_tensor("scratch", [N, N], f16, kind="ExternalOutput").ap()

    with tile.TileContext(nc) as tc:
        with tc.tile_pool(name="const", bufs=1) as cp, \
             tc.tile_pool(name="work", bufs=3) as wp, \
             tc.tile_pool(name="outp", bufs=3) as op, \
             tc.tile_pool(name="pss", bufs=2, space="PSUM") as pss, \
             tc.tile_pool(name="psm", bufs=2, space="PSUM") as psm:

            t_sv_lhsT = cp.tile([15, NB * 128], bf16, tag="t_sv_lhsT")
            nc.sync.dma_start(t_sv_lhsT[:], sv_lhsT)
            t_sv_rhs = cp.tile([15, N], bf16, tag="t_sv_rhs")
            nc.sync.dma_start(t_sv_rhs[:], sv_rhs)
            t_g = cp.tile([128, N], f16, tag="t_g")
            nc.sync.dma_start(t_g[:], gtab)
            t_sa = cp.tile([128, NB], f32, tag="t_sa")
            nc.sync.dma_start(t_sa[:], salpha)
            t_cl = cp.tile([128, 1], f32, tag="t_cl")
            nc.sync.dma_start(t_cl[:], clamp)
            t_mixw = cp.tile([128, 128], f16, tag="t_mixw")
            nc.sync.dma_start(t_mixw[:], mixw)

            # Software-pipelined schedule. Per loop step t:
            #   A(t)   : S~ matmuls              [PE]
            #   B(t-1) : sqrt                    [ACT]
            #   C(t-1) : sub + fused add/max     [DVE (+Pool share)]
            #   E(t-2) : psum->f16 copies + DMA  [ACT+DVE, SP]
            #   D(t-1) : mix matmuls             [PE]
            # Per-engine queue order is what matters: sqrt(k) precedes
            # copy(k-1) on ACT (and sub/ts(k) precede copyD(k-1) on DVE), so
            # the long chain sqrt->sub->ts->mix->copy never serializes two
            # iterations back-to-back.  E(k-1) is still emitted before D(k)
            # so the single-buffered mix psum gets a correct WAR dependency.
            iters = [(b, jp) for b in range(NB) for jp in range(NJ)]
            ITERS = len(iters)
            sA = {}
            sC = {}
            sD = {}

            for t in range(ITERS + 2):
                if t < ITERS:
                    b, jp = iters[t]
                    j0 = jp * JP
                    ps_s = pss.tile([128, JP], f32, tag="ps_s")
                    for h in range(2):
                        nc.tensor.matmul(
                            ps_s[:, h * 512:(h + 1) * 512],
                            t_sv_lhsT[:, b * 128:(b + 1) * 128],
                            t_sv_rhs[:, j0 + h * 512:j0 + (h + 1) * 512],
                            start=True, stop=True,
                        )
                    sA[t] = ps_s

                k = t - 1
                if 0 <= k < ITERS:
                    b, jp = iters[k]
                    j0 = jp * JP
                    ps_s = sA.pop(k)

                    t_d = wp.tile([128, JP], f16, tag="t_d")
                    nc.scalar.activation(
                        t_d[:], ps_s[:], mybir.ActivationFunctionType.Sqrt
                    )

                    t_t = wp.tile([128, JP], f16, tag="t_t")
                    gsl = t_g[:, j0:j0 + JP]
                    if sigma > 0:
                        in0, in1 = t_d[:], gsl
                    else:
                        in0, in1 = gsl, t_d[:]
                    if POOL_TT > 0:
                        nc.gpsimd.tensor_tensor(
                            t_t[:, 0:POOL_TT], in0[:, 0:POOL_TT],
                            in1[:, 0:POOL_TT], AL.subtract,
                        )
                    nc.vector.tensor_tensor(
                        t_t[:, POOL_TT:JP], in0[:, POOL_TT:JP],
                        in1[:, POOL_TT:JP], AL.subtract,
                    )

                    t_h = wp.tile([128, JP], f16, tag="t_h")
                    ts_eng = nc.gpsimd if POOL_TS else nc.vector
                    ts_eng.tensor_scalar(
                        t_h[:], t_t[:], t_sa[:, b:b + 1], t_cl[:, 0:1],
                        AL.add, AL.max,
                    )
                    sC[k] = t_h

                k = t - 2
                if k >= 0:
                    ps_m0, ps_m1, b, jp = sD.pop(k)
                    t_o = op.tile([128, 2 * JP], f16, tag="t_o")
                    nc.scalar.copy(t_o[:, 0:JP], ps_m0[:])
                    nc.vector.tensor_copy(t_o[:, JP:2 * JP], ps_m1[:])
                    # scratch rows r = 256*b + 128*w + q <-> sbuf [q, w*JP+j]
                    row0 = b * IB * DIM_OUT
                    j0 = jp * JP
                    dview = scratch[row0:row0 + 256, j0:j0 + JP].rearrange(
                        "(w q) j -> q w j", w=2
                    )
                    nc.sync.dma_start(
                        dview, t_o[:].rearrange("q (w j) -> q w j", w=2)
                    )

                k = t - 1
                if 0 <= k < ITERS:
                    b, jp = iters[k]
                    t_h = sC.pop(k)
                    ps_mw = []
                    for w in range(2):
                        pr = slice(64 * w, 64 * w + 64)
                        ps_m = psm.tile([128, JP], f32, tag="ps_m")
                        for jh in range(2):
                            nc.tensor.matmul(
                                ps_m[:, jh * 512:(jh + 1) * 512],
                                t_mixw[pr, :],
                                t_h[pr, jh * 512:(jh + 1) * 512],
                                start=True, stop=True,
                            )
                        ps_mw.append(ps_m)
                    sD[k] = (ps_mw[0], ps_mw[1], b, jp)

    nc.compile()
    return nc


def _host_inputs(node_coords, W1, b1, W2, b2, sigma, mprime):
    """Build per-core input maps (small host-side numpy work)."""
    coords = node_coords.astype(np.float32)
    W1 = W1.astype(np.float32)
    b1 = b1.astype(np.float32)
    W2 = W2.astype(np.float32)

    a = W1[:, 3]                       # [3] dist coefficients
    s = np.where(a < 0, -1.0, 1.0).astype(np.float32)
    a2 = a * a
    Wc = W1[:, :3]                     # [3,3] coord coefficients
    g = coords @ Wc.T                  # [N,3]  g_m(j)
    c2 = (coords * coords).sum(1)      # [N]

    # slot -> (unit index, is_pad)
    slot_m = [0, 1, 2, mprime if mprime is not None else 0]

    # ---- g table [128, N] f16: row p=4*di+m -> s_m * g_m(j) ----
    gtab = np.zeros((128, N), np.float32)
    for p in range(128):
        m = slot_m[p % 4]
        gtab[p] = s[m] * g[:, m]
    if mprime is None:
        gtab[3::4] = 0.0

    # ---- clamp [128,1]: 0 on unit slots (relu), -BIG on pad slot ----
    clamp = np.zeros((128, 1), np.float32)
    if mprime is not None:
        clamp[3::4, 0] = NEG_BIG

    # ---- mix weights [128, 128] f16 (both 64-row windows identical) ----
    mixw = np.zeros((128, 128), np.float32)
    for w in range(2):
        for dl in range(16):
            for m in range(3):
                mixw[64 * w + 4 * dl + m, 8 * dl:8 * dl + 8] = W2[:, m]
            if mprime is not None:
                mixw[64 * w + 4 * dl + 3, 8 * dl:8 * dl + 8] = -W2[:, mprime]

    # ---- S~ rhs basis [5, N]: [c_x, c_y, c_z, |c|^2, 1] ----
    s_base_r = np.zeros((5, N), np.float32)
    s_base_r[0:3] = coords.T
    s_base_r[3] = c2
    s_base_r[4] = 1.0
    Rh, Rl = _split2(s_base_r)
    sv_rhs = np.vstack([Rh, Rl, Rh])                  # [15, N]

    in_maps = []
    for c in range(N_CORES):
        r = coords[c * ROWS:(c + 1) * ROWS]          # [512,3]
        r2 = (r * r).sum(1)                          # [512]
        alpha = r @ Wc.T + b1                        # [512,3]

        # ---- S~ lhsT basis [5, NB*128]: col = b*128 + 4*di + m ----
        s_base_l = np.zeros((5, NB * 128), np.float32)
        salpha = np.zeros((128, NB), np.float32)
        i_idx = np.arange(ROWS)
        col = (i_idx // IB) * 128 + 4 * (i_idx % IB)  # base col (slot 0)
        for sl in range(4):
            m = slot_m[sl]
            A = a2[m]
            if mprime is None and sl == 3:
                # harmless constant slot: S~ = 1, no g/alpha, mix col = 0
                s_base_l[4, col + sl] = 1.0
                continue
            cm = col + sl
            s_base_l[0:3, cm] = (-2.0 * A) * r.T
            s_base_l[3, cm] = A
            s_base_l[4, cm] = A * (r2 + EPS)
            # per-partition alpha scalar: sigma * s_m * alpha_m(i), laid
            # out [128, NB] with row 4*di+sl, column b
            sa = sigma * s[m] * alpha[:, m]          # [512]
            salpha[4 * (i_idx % IB) + sl, i_idx // IB] = sa

        Lh, Ll = _split2(s_base_l)
        sv_lhsT = np.vstack([Lh, Lh, Ll])             # [15, 2048]

        in_maps.append({
            "sv_lhsT": np.ascontiguousarray(sv_lhsT),
            "sv_rhs": np.ascontiguousarray(sv_rhs),
            "gtab": gtab.astype(_F16),
            "salpha": salpha,
            "clamp": clamp,
            "mixw": mixw.astype(_F16),
        })
    return in_maps


def kernel(node_coords, W1, b1, W2, b2):
    from concourse.bass_utils import run_bass_kernel_spmd

    node_coords = np.asarray(node_coords)
    W1 = np.asarray(W1)
    b1 = np.asarray(b1)
    W2 = np.asarray(W2)
    b2 = np.asarray(b2)

    a = W1.astype(np.float32)[:, 3]
    s = np.where(a < 0, -1.0, 1.0)
    sigma = 1.0 if s.sum() > 0 else -1.0
    mins = np.nonzero(s != sigma)[0]
    mprime = int(mins[0]) if len(mins) else None

    key = ("nc", sigma)
    if key not in _CACHE:
        _CACHE[key] = _build_program(sigma)
    nc = _CACHE[key]

    in_maps = _host_inputs(node_coords, W1, b1, W2, b2, sigma, mprime)
    res = run_bass_kernel_spmd(nc, in_maps, core_ids=list(range(N_CORES)))
    _CACHE["last_res"] = res

    out = np.empty((N, N, DIM_OUT), np.float32)
    for c in range(N_CORES):
        sc = np.asarray(res.results[c]["scratch"])       # [4096, 4096] f16
        blk = sc.reshape(ROWS, DIM_OUT, N).transpose(0, 2, 1)
        out[c * ROWS:(c + 1) * ROWS] = blk

    # b2 is handled here (the device mix omits it)
    b2f = b2.astype(np.float32)
    if np.any(b2f):
        out += b2f

    # exact diagonal (pairwise features are exactly zero there; the device
    # path has an eps floor under the sqrt, so patch on host)
    h_diag = np.maximum(b1.astype(np.float32), 0.0)
    diag = W2.astype(np.float32) @ h_diag + b2f
    idx = np.arange(N)
    out[idx, idx, :] = diag

    return out


# revision 13
# speedup vs baseline: 5.2528x; 5.2528x over previous
"""Distance-aware masking kernel for Trainium2 (8 NeuronCores).

Computes mask[i,j,:] = W2 @ relu(W1 @ [r_i - c_j, |r_i - c_j|] + b1) + b2
for N=4096 nodes, DIM_OUT=8, sharded by rows across 8 cores (512 rows each).

Math (per hidden unit m, a_m = W1[m,3], s_m = sign(a_m)):
  u_m(i,j) = a_m*dist + alpha_m(i) - g_m(j),  h_m = relu(u_m)
with alpha = r@Wc.T + b1, g = c@Wc.T.  Let sigma = majority sign of a.
Define the uniform slot value (computable with ONE tensor_tensor subtract
whose operand order depends on sigma, plus per-partition scalars):
  t_p = sigma*(|a_m| D) + s_m*... arranged so that slots with s_m == sigma
  hold u_m and the (<=1) minority slot holds -u_m'.
Then relu is a single fused tensor_scalar (add alpha-scalar, max clamp):
clamp=0 gives relu(u) / relu(-u); the pad slot (clamp=-BIG) passes -u_m'
linearly and the block-diagonal mix matmul adds W2[:,m']*u_m' via a
-W2[:,m'] column, using relu(u) = u + relu(-u).

Per-core pipeline, 64 iterations of (block b: 32 rows) x (jp: 1024 cols):
  PE   : S~ = a_m^2 (dist^2+eps) as K=15 bf16 split-pair matmul -> PSUM
  ACT  : t_d = sqrt(S~) -> f16 SBUF  (+ share of PSUM->SBUF output copies)
  DVE  : tmp = t_d - s*g (f16 2x) ; h = max(tmp + s_alpha, clamp) (f16 4x)
  Pool : share of the tmp subtract (SBUF-only engine)
  PE   : 4 block-diagonal mix matmuls (f16) -> PSUM [q=8*dl+o, (w,j)]
  ACT/DVE: copy PSUM f32 -> f16 SBUF, DMA out (f16 scratch, 2KB chunks)
Host: assemble f32 output, add b2, patch exact diagonal.
"""

import sys

sys.path.insert(0, "/opt/trn_rl_repo")

import numpy as np
import ml_dtypes

N = 4096
N_CORES = 8
ROWS = N // N_CORES          # 512 rows per core
IB = 32                      # i-rows per block (x4 slots = 128 partitions)
NB = ROWS // IB              # 16 blocks
JP = 1024                    # j-columns per iteration
NJ = N // JP                 # 4 j-chunks
EPS = 2e-3                   # dist^2 floor; covers bf16-split basis error
DIM = 3
DIM_OUT = 8
NEG_BIG = -3.0e38            # clamp sentinel: pass-through (no relu)

POOL_TT = 0                  # columns of the subtract done on GpSimd/Pool
POOL_TS = False              # run the fused add/max on GpSimd/Pool

_BF = ml_dtypes.bfloat16
_F16 = np.float16

_CACHE = {}


def _split2(x):
    hi = x.astype(_BF)
    lo = (x - hi.astype(np.float32)).astype(_BF)
    return hi, lo


def _build_program(sigma):
    """Build + compile the SPMD Bass program (sigma = majority sign of a)."""
    import concourse.bass as bass  # noqa: F401
    import concourse.mybir as mybir
    import concourse.tile as tile
    from concourse import bacc

    nc = bacc.Bacc("TRN2", target_bir_lowering=False, num_devices=N_CORES)

    f32 = mybir.dt.float32
    f16 = mybir.dt.float16
    bf16 = mybir.dt.bfloat16
    AL = mybir.AluOpType

    sv_lhsT = nc.dram_tensor("sv_lhsT", [15, NB * 128], bf16, kind="ExternalInput").ap()
    sv_rhs = nc.dram_tensor("sv_rhs", [15, N], bf16, kind="ExternalInput").ap()
    gtab = nc.dram_tensor("gtab", [128, N], f16, kind="ExternalInput").ap()
    salpha = nc.dram_tensor("salpha", [128, NB], f32, kind="ExternalInput").ap()
    clamp = nc.dram_tensor("clamp", [128, 1], f32, kind="ExternalInput").ap()
    mixw = nc.dram_tensor("mixw", [128, 128], f16, kind="ExternalInput").ap()
    scratch = nc.dram_tensor("scratch", [N, N], f16, kind="ExternalOutput").ap()

    with tile.TileContext(nc) as tc:
        with tc.tile_pool(name="const", bufs=1) as cp, \
             tc.tile_pool(name="work", bufs=4) as wp, \
             tc.tile_pool(name="outp", bufs=4) as op, \
             tc.tile_pool(name="pss", bufs=2, space="PSUM") as pss, \
             tc.tile_pool(name="psm", bufs=2, space="PSUM") as psm:

            # load order: matmul operands first (unblock the S matmuls),
            # the big g table last (first consumed by the sub, ~2 stages in)
            t_sv_lhsT = cp.tile([15, NB * 128], bf16, tag="t_sv_lhsT")
            nc.sync.dma_start(t_sv_lhsT[:], sv_lhsT)
            t_sv_rhs = cp.tile([15, N], bf16, tag="t_sv_rhs")
            nc.sync.dma_start(t_sv_rhs[:], sv_rhs)
            t_mixw = cp.tile([128, 128], f16, tag="t_mixw")
            nc.sync.dma_start(t_mixw[:], mixw)
            t_sa = cp.tile([128, NB], f32, tag="t_sa")
            nc.sync.dma_start(t_sa[:], salpha)
            t_cl = cp.tile([128, 1], f32, tag="t_cl")
            nc.sync.dma_start(t_cl[:], clamp)
            t_g = cp.tile([128, N], f16, tag="t_g")
            nc.sync.dma_start(t_g[:], gtab)

            # Software-pipelined schedule. Per loop step t:
            #   A(t)   : S~ matmuls              [PE]
            #   B(t-1) : sqrt                    [ACT]
            #   C(t-1) : sub + fused add/max     [DVE (+Pool share)]
            #   E(t-2) : psum->f16 copies + DMA  [ACT+DVE, SP]
            #   D(t-1) : mix matmuls             [PE]
            # Per-engine queue order is what matters: sqrt(k) precedes
            # copy(k-1) on ACT (and sub/ts(k) precede copyD(k-1) on DVE), so
            # the long chain sqrt->sub->ts->mix->copy never serializes two
            # iterations back-to-back.  E(k-1) is still emitted before D(k)
            # so the single-buffered mix psum gets a correct WAR dependency.
            iters = [(b, jp) for b in range(NB) for jp in range(NJ)]
            ITERS = len(iters)
            sA = {}
            sC = {}
            sD = {}

            for t in range(ITERS + 2):
                if t < ITERS:
                    b, jp = iters[t]
                    j0 = jp * JP
                    ps_s = pss.tile([128, JP], f32, tag="ps_s")
                    for h in range(2):
                        nc.tensor.matmul(
                            ps_s[:, h * 512:(h + 1) * 512],
                            t_sv_lhsT[:, b * 128:(b + 1) * 128],
                            t_sv_rhs[:, j0 + h * 512:j0 + (h + 1) * 512],
                            start=True, stop=True,
                        )
                    sA[t] = ps_s

                k = t - 1
                if 0 <= k < ITERS:
                    b, jp = iters[k]
                    j0 = jp * JP
                    ps_s = sA.pop(k)

                    t_d = wp.tile([128, JP], f16, tag="t_d")
                    nc.scalar.activation(
                        t_d[:], ps_s[:], mybir.ActivationFunctionType.Sqrt
                    )

                    t_t = wp.tile([128, JP], f16, tag="t_t")
                    gsl = t_g[:, j0:j0 + JP]
                    if sigma > 0:
                        in0, in1 = t_d[:], gsl
                    else:
                        in0, in1 = gsl, t_d[:]
                    if POOL_TT > 0:
                        nc.gpsimd.tensor_tensor(
                            t_t[:, 0:POOL_TT], in0[:, 0:POOL_TT],
                            in1[:, 0:POOL_TT], AL.subtract,
                        )
                    nc.vector.tensor_tensor(
                        t_t[:, POOL_TT:JP], in0[:, POOL_TT:JP],
                        in1[:, POOL_TT:JP], AL.subtract,
                    )

                    t_h = wp.tile([128, JP], f16, tag="t_h")
                    ts_eng = nc.gpsimd if POOL_TS else nc.vector
                    ts_eng.tensor_scalar(
                        t_h[:], t_t[:], t_sa[:, b:b + 1], t_cl[:, 0:1],
                        AL.add, AL.max,
                    )
                    sC[k] = t_h

                k = t - 2
                if k >= 0:
                    ps_m0, ps_m1, b, jp = sD.pop(k)
                    t_o = op.tile([128, 2 * JP], f16, tag="t_o")
                    nc.scalar.copy(t_o[:, 0:JP], ps_m0[:])
                    nc.vector.tensor_copy(t_o[:, JP:2 * JP], ps_m1[:])
                    # scratch rows r = 256*b + 128*w + q <-> sbuf [q, w*JP+j]
                    row0 = b * IB * DIM_OUT
                    j0 = jp * JP
                    dview = scratch[row0:row0 + 256, j0:j0 + JP].rearrange(
                        "(w q) j -> q w j", w=2
                    )
                    nc.sync.dma_start(
                        dview, t_o[:].rearrange("q (w j) -> q w j", w=2)
                    )

                k = t - 1
                if 0 <= k < ITERS:
                    b, jp = iters[k]
                    t_h = sC.pop(k)
                    ps_mw = []
                    for w in range(2):
                        pr = slice(64 * w, 64 * w + 64)
                        ps_m = psm.tile([128, JP], f32, tag="ps_m")
                        for jh in range(2):
                            nc.tensor.matmul(
                                ps_m[:, jh * 512:(jh + 1) * 512],
                                t_mixw[pr, :],
                                t_h[pr, jh * 512:(jh + 1) * 512],
                                start=True, stop=True,
                            )
                        ps_mw.append(ps_m)
                    sD[k] = (ps_mw[0], ps_mw[1], b, jp)

    nc.compile()
    return nc


def _host_inputs(node_coords, W1, b1, W2, b2, sigma, mprime):
    """Build per-core input maps (small host-side numpy work)."""
    coords = node_coords.astype(np.float32)
    W1 = W1.astype(np.float32)
    b1 = b1.astype(np.float32)
    W2 = W2.astype(np.float32)

    a = W1[:, 3]                       # [3] dist coefficients
    s = np.where(a < 0, -1.0, 1.0).astype(np.float32)
    a2 = a * a
    Wc = W1[:, :3]                     # [3,3] coord coefficients
    g = coords @ Wc.T                  # [N,3]  g_m(j)
    c2 = (coords * coords).sum(1)      # [N]

    # slot -> (unit index, is_pad)
    slot_m = [0, 1, 2, mprime if mprime is not None else 0]

    # ---- g table [128, N] f16: row p=4*di+m -> s_m * g_m(j) ----
    gtab = np.zeros((128, N), np.float32)
    for p in range(128):
        m = slot_m[p % 4]
        gtab[p] = s[m] * g[:, m]
    if mprime is None:
        gtab[3::4] = 0.0

    # ---- clamp [128,1]: 0 on unit slots (relu), -BIG on pad slot ----
    clamp = np.zeros((128, 1), np.float32)
    if mprime is not None:
        clamp[3::4, 0] = NEG_BIG

    # ---- mix weights [128, 128] f16 (both 64-row windows identical) ----
    mixw = np.zeros((128, 128), np.float32)
    for w in range(2):
        for dl in range(16):
            for m in range(3):
                mixw[64 * w + 4 * dl + m, 8 * dl:8 * dl + 8] = W2[:, m]
            if mprime is not None:
                mixw[64 * w + 4 * dl + 3, 8 * dl:8 * dl + 8] = -W2[:, mprime]

    # ---- S~ rhs basis [5, N]: [c_x, c_y, c_z, |c|^2, 1] ----
    s_base_r = np.zeros((5, N), np.float32)
    s_base_r[0:3] = coords.T
    s_base_r[3] = c2
    s_base_r[4] = 1.0
    Rh, Rl = _split2(s_base_r)
    sv_rhs = np.vstack([Rh, Rl, Rh])                  # [15, N]

    in_maps = []
    for c in range(N_CORES):
        r = coords[c * ROWS:(c + 1) * ROWS]          # [512,3]
        r2 = (r * r).sum(1)                          # [512]
        alpha = r @ Wc.T + b1                        # [512,3]

        # ---- S~ lhsT basis [5, NB*128]: col = b*128 + 4*di + m ----
        s_base_l = np.zeros((5, NB * 128), np.float32)
        salpha = np.zeros((128, NB), np.float32)
        i_idx = np.arange(ROWS)
        col = (i_idx // IB) * 128 + 4 * (i_idx % IB)  # base col (slot 0)
        for sl in range(4):
            m = slot_m[sl]
            A = a2[m]
            if mprime is None and sl == 3:
                # harmless constant slot: S~ = 1, no g/alpha, mix col = 0
                s_base_l[4, col + sl] = 1.0
                continue
            cm = col + sl
            s_base_l[0:3, cm] = (-2.0 * A) * r.T
            s_base_l[3, cm] = A
            s_base_l[4, cm] = A * (r2 + EPS)
            # per-partition alpha scalar: sigma * s_m * alpha_m(i), laid
            # out [128, NB] with row 4*di+sl, column b
            sa = sigma * s[m] * alpha[:, m]          # [512]
            salpha[4 * (i_idx % IB) + sl, i_idx // IB] = sa

        Lh, Ll = _split2(s_base_l)
        sv_lhsT = np.vstack([Lh, Lh, Ll])             # [15, 2048]

        in_maps.append({
            "sv_lhsT": np.ascontiguousarray(sv_lhsT),
            "sv_rhs": np.ascontiguousarray(sv_rhs),
            "gtab": gtab.astype(_F16),
            "salpha": salpha,
            "clamp": clamp,
            "mixw": mixw.astype(_F16),
        })
    return in_maps


def kernel(node_coords, W1, b1, W2, b2):
    from concourse.bass_utils import run_bass_kernel_spmd

    node_coords = np.asarray(node_coords)
    W1 = np.asarray(W1)
    b1 = np.asarray(b1)
    W2 = np.asarray(W2)
    b2 = np.asarray(b2)

    a = W1.astype(np.float32)[:, 3]
    s = np.where(a < 0, -1.0, 1.0)
    sigma = 1.0 if s.sum() > 0 else -1.0
    mins = np.nonzero(s != sigma)[0]
    mprime = int(mins[0]) if len(mins) else None

    key = ("nc", sigma)
    if key not in _CACHE:
        _CACHE[key] = _build_program(sigma)
    nc = _CACHE[key]

    in_maps = _host_inputs(node_coords, W1, b1, W2, b2, sigma, mprime)
    res = run_bass_kernel_spmd(nc, in_maps, core_ids=list(range(N_CORES)))
    _CACHE["last_res"] = res

    out = np.empty((N, N, DIM_OUT), np.float32)
    for c in range(N_CORES):
        sc = np.asarray(res.results[c]["scratch"])       # [4096, 4096] f16
        blk = sc.reshape(ROWS, DIM_OUT, N).transpose(0, 2, 1)
        out[c * ROWS:(c + 1) * ROWS] = blk

    # b2 is handled here (the device mix omits it)
    b2f = b2.astype(np.float32)
    if np.any(b2f):
        out += b2f

    # exact diagonal (pairwise features are exactly zero there; the device
    # path has an eps floor under the sqrt, so patch on host)
    h_diag = np.maximum(b1.astype(np.float32), 0.0)
    diag = W2.astype(np.float32) @ h_diag + b2f
    idx = np.arange(N)
    out[idx, idx, :] = diag

    return out
